# revision 11
# baseline (speedup 1.0000x reference)
"""Trainium2 Bass kernel for a 2-layer GCN (NextHopGNN) — bf16 rework.

Distribution: destination-node sharding across 8 NeuronCores. Each core owns
N/8 destination nodes and all edges pointing into them. Per layer:
  1. each core computes its slice of the scaled feature table
     y = dinv * (x @ W) in bf16 (single-pass PE matmuls)
  2. AllGather (bf16) -> every core holds the full [N, 64] table in HBM
  3. per 128-node dst tile: dma_gather the edge source rows. The table is
     viewed as node PAIRS [N/2, 128] bf16 so each 256B gather element holds
     two node rows; edges are bucketed by (dst_tile, src block, src parity)
     and the matmul rhs statically slices the even/odd half. One-hot scatter
     matrices built on the vector engine (bf16), accumulated S^T @ G in PSUM.
     Self-loop handled by appending real self-edges (they mostly occupy
     bucket padding slots); bias via a rank-1 matmul pre-multiplied by
     sqrt(deg) so the tile is finished by one per-partition dinv scale.
Layer-1 epilogue computes the layer-2 table tile (relu -> transpose -> @W2
-> scale) so layer 2 only needs the second AllGather + aggregation.
"""
import sys
import os
import numpy as np

sys.path.insert(0, "/opt/trn_rl_repo")

P = 128
H = 64
EDIM = 128
NCORES = 8
NCLS = 4            # bucket classes per tile: (src_block(2) x parity(2))
NSTREAMS = 2        # gather streams (one per src block of 50000 nodes)
GCHUNK = 8          # max chunks (of 128 idxs) per dma_gather instruction
SBATCH = 8          # chunks per batched one-hot build
DSTW = 64           # dstloc columns per DMA load (multiple of SBATCH)

_COMPILED = {}


def _ceil_div(a, b):
    return (a + b - 1) // b


def make_schedule(edge_index, n_nodes, n_cores=NCORES):
    """Host-side marshaling: shard edges by dst owner, append self-edges,
    bucket by (dst_tile, src_block, src_parity), pad each bucket to chunks
    of 128 with a shared chunk count across cores so all cores run an
    identical program."""
    src = edge_index[0].astype(np.int64)
    dst = edge_index[1].astype(np.int64)
    npc = n_nodes // n_cores
    T = _ceil_div(npc, P)
    BS = n_nodes // NSTREAMS           # nodes per src block (50000)
    PAIRS = BS // 2                    # pair rows per block (25000)
    assert PAIRS < 32768

    counts = np.zeros((n_cores, T, NCLS), np.int64)
    percore = []
    loop = np.arange(npc, dtype=np.int64)
    for c in range(n_cores):
        sel = (dst >= c * npc) & (dst < (c + 1) * npc)
        s = np.concatenate([src[sel], loop + c * npc])   # self-edges appended
        d = np.concatenate([dst[sel] - c * npc, loop])
        t = d >> 7
        q = (s // BS) * 2 + (s & 1)    # class: (block, parity)
        key = t * NCLS + q
        order = np.argsort(key, kind="stable")
        s, d, key = s[order], d[order], key[order]
        cnt = np.bincount(key, minlength=T * NCLS).reshape(T, NCLS)
        counts[c] = cnt
        percore.append((s, d, cnt))

    K = _ceil_div(counts.max(axis=0), P).astype(np.int64)  # [T, NCLS]
    # stream of class q is q>>1; stream chunk counts
    S_b = np.array([K[:, 0:2].sum(), K[:, 2:4].sum()], np.int64)
    total_chunks = int(K.sum())

    # matmul-order index of chunk (t, q); gather-stream position per stream
    m_start = np.zeros((T, NCLS), np.int64)
    pos_q = np.zeros((T, NCLS), np.int64)
    m = 0
    poss = np.zeros(NSTREAMS, np.int64)
    for t in range(T):
        for q in range(NCLS):
            m_start[t, q] = m
            pos_q[t, q] = poss[q >> 1]
            m += K[t, q]
            poss[q >> 1] += K[t, q]

    # per-stream gather instruction sizes (in chunks)
    gather_groups = []
    for b in range(NSTREAMS):
        sizes = []
        rem = int(S_b[b])
        while rem > 0:
            g = min(GCHUNK, rem)
            sizes.append(g)
            rem -= g
        gather_groups.append(sizes)

    # per-core data streams
    core_data = []
    for c in range(n_cores):
        s, d, cnt = percore[c]
        off = np.concatenate([[0], np.cumsum(cnt.reshape(-1))]).astype(np.int64)
        idx_streams = [np.zeros(int(S_b[b]) * P, np.int16) for b in range(NSTREAMS)]
        dstloc = np.full(total_chunks * P, -1.0, np.float32)
        for t in range(T):
            for q in range(NCLS):
                n = int(cnt[t, q])
                if n == 0:
                    continue
                b = q >> 1
                o = off[t * NCLS + q]
                ss = (s[o:o + n] - b * BS) >> 1     # pair-local index
                dd = d[o:o + n] - t * P
                p0 = int(pos_q[t, q]) * P
                idx_streams[b][p0:p0 + n] = ss.astype(np.int16)
                q0 = int(m_start[t, q]) * P
                dstloc[q0:q0 + n] = dd.astype(np.float32)
        # wrap idx streams for the gather ucode: [128, S_b*8] int16
        idx_wrapped = []
        for b in range(NSTREAMS):
            w = idx_streams[b].reshape(-1, 16).T          # [16, S_b*8]
            idx_wrapped.append(np.tile(w, (8, 1)).astype(np.int16))
        dst_t = dstloc.reshape(total_chunks, P).T.copy()  # [128, total_chunks]
        core_data.append((idx_wrapped, dst_t))

    return {
        "n_nodes": n_nodes, "n_cores": n_cores, "npc": npc, "T": T, "BS": BS,
        "PAIRS": PAIRS, "K": K, "S_b": S_b.astype(np.int64),
        "total_chunks": total_chunks, "m_start": m_start, "pos_q": pos_q,
        "gather_groups": gather_groups, "core_data": core_data,
    }


def build_bass(sched):
    from concourse import bass, bacc, tile, mybir

    n_cores = sched["n_cores"]
    npc = sched["npc"]
    T = sched["T"]
    N = sched["n_nodes"]
    BS = sched["BS"]
    PAIRS = sched["PAIRS"]
    K = sched["K"]
    S_b = sched["S_b"]
    total_chunks = sched["total_chunks"]
    m_start = sched["m_start"]
    pos_q = sched["pos_q"]
    gather_groups = sched["gather_groups"]
    f32 = mybir.dt.float32
    bf16 = mybir.dt.bfloat16
    i16 = mybir.dt.int16
    i32 = mybir.dt.int32

    nc = bacc.Bacc("TRN2", target_bir_lowering=False, debug=False,
                   enable_asserts=True, num_devices=n_cores,
                   num_swdge_queues=4)

    embT = nc.dram_tensor("embT", [P, T * P], bf16, kind="ExternalInput")
    W1_d = nc.dram_tensor("W1", [EDIM, H], bf16, kind="ExternalInput")
    W2_d = nc.dram_tensor("W2", [H, H], bf16, kind="ExternalInput")
    b1_d = nc.dram_tensor("b1r", [1, H], f32, kind="ExternalInput")
    b2_d = nc.dram_tensor("b2r", [1, H], f32, kind="ExternalInput")
    dinv_d = nc.dram_tensor("dinv_t", [P, T], f32, kind="ExternalInput")
    sqd_d = nc.dram_tensor("sqd_row", [1, T * P], f32, kind="ExternalInput")
    idx_d = [nc.dram_tensor(f"idx{b}", [P, max(int(S_b[b]), 1) * 8], i16,
                            kind="ExternalInput") for b in range(NSTREAMS)]
    dst_d = nc.dram_tensor("dstloc", [P, max(total_chunks, 1)], f32,
                           kind="ExternalInput")
    out_d = nc.dram_tensor("out", [npc, H], f32, kind="ExternalOutput")

    with tile.TileContext(nc) as tc:
        with tc.tile_pool(name="const", bufs=1) as constp, \
             tc.tile_pool(name="tables", bufs=1) as tablep, \
             tc.tile_pool(name="work", bufs=3) as workp, \
             tc.tile_pool(name="gath", bufs=3) as gathp, \
             tc.tile_pool(name="spool", bufs=3) as spool, \
             tc.tile_pool(name="psum", bufs=4, space="PSUM") as psump, \
             tc.tile_pool(name="psumT", bufs=2, space="PSUM") as psumTp, \
             tc.tile_pool(name="dram", bufs=1, space="DRAM") as dramp:

            # ---- constants ----
            from concourse.masks import make_identity
            ident = constp.tile([P, P], bf16)
            make_identity(nc, ident[:])
            iota_i = constp.tile([P, SBATCH * P], i32)
            nc.gpsimd.iota(iota_i[:], pattern=[[0, SBATCH], [1, P]],
                           base=0, channel_multiplier=0)
            iota_f = constp.tile([P, SBATCH * P], f32)
            nc.vector.tensor_copy(iota_f[:], iota_i[:])

            W1_s = constp.tile([EDIM, H], bf16)
            nc.sync.dma_start(out=W1_s[:], in_=W1_d[:])
            W2_s = constp.tile([H, H], bf16)
            nc.sync.dma_start(out=W2_s[:], in_=W2_d[:])
            b1_s = constp.tile([1, H], f32)
            nc.sync.dma_start(out=b1_s[:], in_=b1_d[:])
            b2_s = constp.tile([1, H], f32)
            nc.sync.dma_start(out=b2_s[:], in_=b2_d[:])
            dinv_s = constp.tile([P, T], f32)
            nc.sync.dma_start(out=dinv_s[:], in_=dinv_d[:])
            sqd_s = constp.tile([1, T * P], f32)
            nc.sync.dma_start(out=sqd_s[:], in_=sqd_d[:])

            # ---- persistent tables in SBUF ----
            y1_all = tablep.tile([P, T * H], bf16)    # layer-1 table, own slice
            y2_all = tablep.tile([P, T * H], bf16)    # layer-2 table, own slice

            # ---- DRAM staging for collectives ----
            y1_in = dramp.tile([npc, H], bf16)
            y1_full = dramp.tile([N, H], bf16, addr_space="Shared")
            y2_in = dramp.tile([npc, H], bf16)
            y2_full = dramp.tile([N, H], bf16, addr_space="Shared")

            # ---- phase 1: y1 = dinv * (emb @ W1) for own nodes ----
            embT_s = tablep.tile([P, T * P], bf16)
            nc.sync.dma_start(out=embT_s[:], in_=embT[:])
            for t in range(T):
                ps = psump.tile([P, H], f32, tag="ps")
                nc.tensor.matmul(ps[:], lhsT=embT_s[:, t * P:(t + 1) * P],
                                 rhs=W1_s[:], start=True, stop=True)
                ys = y1_all[:, t * H:(t + 1) * H]
                nc.vector.tensor_scalar_mul(ys, ps[:], dinv_s[:, t:t + 1])
                rows = min(npc - t * P, P)
                eng = nc.sync if t % 2 == 0 else nc.scalar
                eng.dma_start(out=y1_in[t * P:t * P + rows, :],
                              in_=y1_all[:rows, t * H:(t + 1) * H])

            # ---- phase 2: AllGather layer-1 table ----
            nc.gpsimd.collective_compute(
                "AllGather", mybir.AluOpType.bypass,
                replica_groups=[list(range(n_cores))],
                ins=[y1_in.opt()],
                outs=[y1_full.opt()],
            )

            # idx/dstloc tables (shared by both layers) load during AG-1
            idx_sb = []
            for b in range(NSTREAMS):
                ixt = constp.tile([P, max(int(S_b[b]), 1) * 8], i16,
                                  name=f"idxsb{b}")
                nc.sync.dma_start(out=ixt[:], in_=idx_d[b][:])
                idx_sb.append(ixt)
            dst_sb = constp.tile([P, max(total_chunks, 1)], f32)
            nc.sync.dma_start(out=dst_sb[:], in_=dst_d[:])

            # ---- aggregation pass (used for both layers) ----
            def aggregation(src_table, b_s, layer):
                # pair view of the table: [N/2, 128] bf16
                pairs = src_table.rearrange("(k two) h -> k (two h)", two=2)
                # per-stream gather bookkeeping
                next_group = [0] * NSTREAMS    # next gather group per stream
                group_start = [0] * NSTREAMS   # stream chunk idx of group start
                gbufs = [None] * NSTREAMS
                gather_ctr = [0]               # cycles the 4 SWDGE queues
                sbuf_tile = [None]             # current one-hot batch tile
                sbatch_lo = [-1]

                def ensure_gather(b, pos):
                    while gbufs[b] is None or pos >= group_start[b] + gbufs[b][1]:
                        g = next_group[b]
                        sizes = gather_groups[b]
                        start = sum(sizes[:g])
                        size = sizes[g]
                        gt = gathp.tile([P, GCHUNK, 2 * H], bf16, tag=f"g{b}")
                        nc.gpsimd.dma_gather(
                            out_ap=gt[:, :size, :],
                            in_ap=pairs[b * PAIRS:(b + 1) * PAIRS, :],
                            idxs_ap=idx_sb[b][:, start * 8:(start + size) * 8],
                            num_idxs=size * P,
                            num_idxs_reg=size * P,
                            elem_size=2 * H,
                            queue_num=gather_ctr[0] % 4,
                        )
                        gather_ctr[0] += 1
                        gbufs[b] = (gt, size)
                        group_start[b] = start
                        next_group[b] += 1
                    return gbufs[b][0]

                def ensure_s(m):
                    lo = (m // SBATCH) * SBATCH
                    if sbatch_lo[0] != lo:
                        nb = min(SBATCH, total_chunks - lo)
                        st = spool.tile([P, SBATCH * P], bf16, tag="S")
                        dl = dst_sb[:, lo:lo + nb]
                        dl3 = dl.rearrange("p (c u) -> p c u", u=1)
                        nc.vector.tensor_tensor(
                            out=st[:, :nb * P].rearrange("p (c j) -> p c j", j=P),
                            in0=iota_f[:, :nb * P].rearrange("p (c j) -> p c j", j=P),
                            in1=dl3.to_broadcast([P, nb, P]),
                            op=mybir.AluOpType.is_equal)
                        sbuf_tile[0] = st
                        sbatch_lo[0] = lo
                    return sbuf_tile[0][:, (m - sbatch_lo[0]) * P:
                                        (m - sbatch_lo[0] + 1) * P]

                for t in range(T):
                    ps = psump.tile([P, H], f32, tag="ps")
                    first = True
                    for q in range(NCLS):
                        b = q >> 1
                        half = q & 1
                        for k in range(int(K[t, q])):
                            pos = int(pos_q[t, q]) + k
                            m = int(m_start[t, q]) + k
                            gt = ensure_gather(b, pos)
                            gview = gt[:, pos - group_start[b],
                                       half * H:(half + 1) * H]
                            sview = ensure_s(m)
                            nc.tensor.matmul(ps[:], lhsT=sview, rhs=gview,
                                             start=first, stop=False)
                            first = False
                    # bias premultiplied by sqrt(deg): psum += sqd_j * b_d
                    nc.tensor.matmul(ps[:], lhsT=sqd_s[:, t * P:(t + 1) * P],
                                     rhs=b_s[:], start=first, stop=True)
                    yield t, ps

            # ---- phase 3: layer-1 aggregation + fused layer-2 table ----
            for t, ps in aggregation(y1_full, b1_s, 1):
                h1 = workp.tile([P, H], bf16, tag="h1")
                nc.scalar.activation(h1[:], ps[:],
                                     mybir.ActivationFunctionType.Relu,
                                     scale=dinv_s[:, t:t + 1])
                pT = psumTp.tile([H, P], bf16)
                nc.tensor.transpose(pT[:], h1[:], ident[:])
                h1T = workp.tile([H, P], bf16, tag="h1T")
                nc.vector.tensor_copy(h1T[:], pT[:])
                ps2 = psump.tile([P, H], f32, tag="ps")
                nc.tensor.matmul(ps2[:], lhsT=h1T[:], rhs=W2_s[:],
                                 start=True, stop=True)
                y2s = y2_all[:, t * H:(t + 1) * H]
                nc.vector.tensor_scalar_mul(y2s, ps2[:], dinv_s[:, t:t + 1])
                rows = min(npc - t * P, P)
                eng = nc.sync if t % 2 == 0 else nc.scalar
                eng.dma_start(out=y2_in[t * P:t * P + rows, :],
                              in_=y2_all[:rows, t * H:(t + 1) * H])

            # ---- phase 4: AllGather layer-2 table ----
            nc.gpsimd.collective_compute(
                "AllGather", mybir.AluOpType.bypass,
                replica_groups=[list(range(n_cores))],
                ins=[y2_in.opt()],
                outs=[y2_full.opt()],
            )

            # ---- phase 5: layer-2 aggregation -> output ----
            for t, ps in aggregation(y2_full, b2_s, 2):
                ot = workp.tile([P, H], f32, tag="ot")
                nc.vector.tensor_scalar_mul(ot[:], ps[:], dinv_s[:, t:t + 1])
                rows = min(npc - t * P, P)
                eng = nc.sync if t % 2 == 0 else nc.scalar
                eng.dma_start(out=out_d[t * P:t * P + rows, :],
                              in_=ot[:rows, :])

    nc.compile()
    return nc


def make_inputs(sched, emb_weight, W1, b1, W2, b2, deg):
    """Build per-core input maps."""
    import ml_dtypes
    bf16 = ml_dtypes.bfloat16
    n_cores = sched["n_cores"]
    npc = sched["npc"]
    T = sched["T"]
    dinv = (1.0 / np.sqrt(deg.astype(np.float64))).astype(np.float32)
    sqd = np.sqrt(deg.astype(np.float64)).astype(np.float32)
    in_maps = []
    for c in range(n_cores):
        lo, hi = c * npc, (c + 1) * npc
        embT = np.zeros((P, T * P), bf16)
        embT[:, :npc] = emb_weight[lo:hi].T.astype(bf16)
        tmp = np.zeros(T * P, np.float32)
        tmp[:npc] = dinv[lo:hi]
        dinv_t = np.ascontiguousarray(tmp.reshape(T, P).T)
        sqd_row = np.zeros((1, T * P), np.float32)
        sqd_row[0, :npc] = sqd[lo:hi]
        idx_wrapped, dst_t = sched["core_data"][c]
        m = {
            "embT": embT,
            "W1": W1.astype(bf16),
            "W2": W2.astype(bf16),
            "b1r": b1.reshape(1, -1).astype(np.float32),
            "b2r": b2.reshape(1, -1).astype(np.float32),
            "dinv_t": dinv_t,
            "sqd_row": sqd_row,
            "dstloc": dst_t,
        }
        for b in range(NSTREAMS):
            iw = idx_wrapped[b]
            if iw.shape[1] == 0:
                iw = np.zeros((P, 8), np.int16)
            m[f"idx{b}"] = iw
        in_maps.append(m)
    return in_maps


def run(edge_index, emb_weight, W1, b1, W2, b2, n_nodes=None, trace=False):
    from concourse import bass_utils
    n_nodes = n_nodes if n_nodes is not None else emb_weight.shape[0]
    sched = make_schedule(np.asarray(edge_index), n_nodes)
    key = ("gnn-v16", n_nodes, int(sched["total_chunks"]),
           tuple(int(x) for x in sched["S_b"]))
    if key not in _COMPILED:
        _COMPILED[key] = build_bass(sched)
    nc = _COMPILED[key]
    deg = np.bincount(np.asarray(edge_index)[1], minlength=n_nodes).astype(np.float32) + 1.0
    in_maps = make_inputs(sched, np.asarray(emb_weight), np.asarray(W1),
                          np.asarray(b1), np.asarray(W2), np.asarray(b2), deg)
    res = bass_utils.run_bass_kernel_spmd(
        nc, in_maps, core_ids=list(range(sched["n_cores"])), trace=trace)
    npc = sched["npc"]
    out = np.concatenate([res.results[c]["out"] for c in range(sched["n_cores"])],
                         axis=0)
    return out[:n_nodes], res


def kernel(edge_index, emb_weight, W1, b1, W2, b2):
    out, _ = run(edge_index, emb_weight, W1, b1, W2, b2)
    return out
